# revision 26
# baseline (speedup 1.0000x reference)
"""Causal multi-head self-attention on 8 Trainium2 NeuronCores (Bass/Tile).

Problem (hardcoded shapes): x [2, 2048, 768] f32, 12 heads of dim 64.
    qkv = x @ Wqkv + bqkv ; per-head causal softmax(q k^T / 8) @ v ; out @ Wproj + bproj

Sharding: 8 cores = 2 batches x 4 head-groups (3 heads each). Each core computes
its heads' QKV, attention, and a partial output projection (its rows of Wproj).
Host sums the 4 partial projections per batch and adds bproj.

All matmul operands are bf16 (PSUM accumulation stays fp32); the 2e-2 rel-err
budget has ~20x headroom over the ~1e-3 this costs. bf16 enables FWL weight
loads and removes the fp32r staging/rounding passes entirely.

Single-pass structure over four 512-column blocks keeps the PE HAM-warm
(no >3.4us tensor-engine idle => stays at 2.4 GHz):
  per block b: DMA xT slice -> qkT chunks -> v tiles -> attention qc=b
               (scores S^T / exp / causal tri / AV, software-pipelined one kk
               deep against the ACT exp) -> normalize -> projection + DMA out.

Layouts per core (no on-device transposes needed anywhere):
  xT[128,6,2048]   x^T, host-pre-transposed, bf16
  qkc[128,3,2048]  [q0q1 | q2k0 | k1k2]^T, heads on 64-row slices (scale 1/8
                   folded into Wq on host)
  vsb[128,16,3,65] v natural + per-key mask column (ones here) for the
                   softmax denominator via the augmented-row trick
  S^T blocks [128 keys, <=512 queries] in PSUM; exp -> bf16 pt tiles; the
  key attn_mask is folded into vsb (mask*v rows and mask sum-column), which
  is exactly equivalent to -inf score masking, so exp needs no bias at all.
  oacc[65,512] fp32 accumulates v_aug^T P^T over key tiles; row 64 = softmax
  denominator; normalize via fast reciprocal + gpsimd partition broadcast.
  y = A @ Wproj emitted bf16; host sums the 4 partials per batch in fp32.

PSUM budget (8 banks): s2/proj pool [128,1024]x2 = 4, aux [128,512]x1 = 1,
oacc [65,512]x3 = 3.
"""
import os
import numpy as np

import concourse.bass as bass
import concourse.mybir as mybir
import concourse.tile as tile
from concourse import bacc
from concourse.bass_utils import run_bass_kernel_spmd
from concourse.masks import make_upper_triangular

f32 = mybir.dt.float32
bf16 = mybir.dt.bfloat16

T = 2048          # sequence length
H = 768           # model dim
NH_CORE = 3       # heads per core
HD = 64           # head dim
NT = T // 128     # 16 seq tiles
KH = H // 128     # 6 contraction chunks for H
NB = 4            # 512-wide column blocks

_cache = {}
last_results = None


def _build():
    """Build the single-core Tile program (same program on all 8 cores)."""
    nc = bacc.Bacc("TRN2", target_bir_lowering=False, debug=False)

    x_d = nc.dram_tensor("xbT", [H, T], bf16, kind="ExternalInput")
    # weights host-prepacked to [128, ...] so each DMA line is one contiguous
    # multi-KB run per partition (row-major [H, N] layouts generate 768-byte
    # descriptors that clog the queues for ~25us at startup)
    wqk_d = nc.dram_tensor("wqk", [128, KH, 384], bf16, kind="ExternalInput")
    wv_d = nc.dram_tensor("wv", [128, KH, 192], bf16, kind="ExternalInput")
    wp_d = nc.dram_tensor("wp", [128, 2, H], bf16, kind="ExternalInput")
    bqk_d = nc.dram_tensor("bqk", [128, 3], f32, kind="ExternalInput")
    bv_d = nc.dram_tensor("bv", [1, 192], bf16, kind="ExternalInput")
    mask_d = nc.dram_tensor("maskt", [128, NT], f32, kind="ExternalInput")
    y_d = nc.dram_tensor("y", [T, H], bf16, kind="ExternalOutput")

    Exp = mybir.ActivationFunctionType.Exp

    with tile.TileContext(nc) as tc:
        with (
            tc.tile_pool(name="singles", bufs=1) as singles,
            tc.tile_pool(name="big", bufs=1) as big,
            tc.tile_pool(name="pt", bufs=4) as ptp,
            tc.tile_pool(name="yout", bufs=2) as yout,
            tc.tile_pool(name="small", bufs=2) as small,
            tc.tile_pool(name="mm", bufs=2, space="PSUM") as mmp,
            tc.tile_pool(name="aux", bufs=1, space="PSUM") as auxp,
            tc.tile_pool(name="acc", bufs=3, space="PSUM") as accp,
        ):
            # ---- constants / weights ----
            tri_f = singles.tile([128, 128], f32)
            make_upper_triangular(nc, tri_f[:], val=1.0, diag=True)
            tri01 = singles.tile([128, 128], bf16)
            nc.vector.tensor_copy(tri01[:], tri_f[:])
            ones1 = singles.tile([1, 128], bf16)
            nc.vector.memset(ones1[:], 1.0)

            xT = big.tile([128, KH, T], bf16)
            # interleave wqk / xT-block0 loads per k-chunk so the first qkT
            # matmul only waits for ~0.2MB of DMA, not the full weight set
            wqk = singles.tile([128, KH, 384], bf16)
            nc.sync.dma_start(out=wqk[:], in_=wqk_d.ap())
            for k in range(KH):
                nc.sync.dma_start(out=xT[:, k, 0:512], in_=x_d.ap()[k * 128:(k + 1) * 128, 0:512])
            bqk = singles.tile([128, 3], f32)
            nc.sync.dma_start(out=bqk[:], in_=bqk_d.ap())
            wv = singles.tile([128, KH, 192], bf16)
            nc.sync.dma_start(out=wv[:], in_=wv_d.ap())
            bv = singles.tile([1, 192], bf16)
            nc.sync.dma_start(out=bv[:], in_=bv_d.ap())
            maskt = singles.tile([128, NT], f32)
            nc.sync.dma_start(out=maskt[:], in_=mask_d.ap())
            wpb = singles.tile([128, 2, H], bf16)
            nc.sync.dma_start(out=wpb[:], in_=wp_d.ap())
            wp0 = wpb[:, 0, :]
            wp1 = wpb[0:64, 1, :]
            qkc = big.tile([128, 3, T], bf16)
            vsb = big.tile([128, NT, NH_CORE, HD + 1], bf16)
            atA = big.tile([128, T], bf16)   # A^T for h0 (0:64), h1 (64:128)
            atB = big.tile([64, T], bf16)    # h2
            at_of = [(atA, 0), (atA, 64), (atB, 0)]
            # chunks: c0=[q0|q1], c1=[k0|k1], c2=[q2|k2]; matmul requires
            # lhsT/rhs base partitions to match, so k2 gets realigned to
            # base 0 via a small per-block copy into qkE.
            qkE = big.tile([64, T], bf16)
            qTs = [qkc[0:64, 0, :], qkc[64:128, 0, :], qkc[0:64, 2, :]]
            kTs = [qkc[0:64, 1, :], qkc[64:128, 1, :], qkE[:, :]]

            def emit_xdma(blk):
                bs = slice(blk * 512, (blk + 1) * 512)
                for k in range(KH):
                    nc.sync.dma_start(out=xT[:, k, bs],
                                      in_=x_d.ap()[k * 128:(k + 1) * 128, bs])

            def emit_qkT(blk):
                bs = slice(blk * 512, (blk + 1) * 512)
                # qkT chunks [384, 512]
                for c in range(3):
                    ps = auxp.tile([128, 512], f32, tag="aux", name="qkps")
                    for k in range(KH):
                        nc.tensor.matmul(ps[:], lhsT=wqk[:, k, c * 128:(c + 1) * 128],
                                         rhs=xT[:, k, bs], start=(k == 0), stop=(k == KH - 1))
                    nc.vector.tensor_scalar_add(qkc[:, c, bs], ps[:], bqk[:, c:c + 1])
                nc.vector.tensor_copy(qkE[:, bs], qkc[64:128, 2, bs])

            def emit_v(blk):
                # v natural for 4 t-tiles, mask folded in
                for t in range(4 * blk, 4 * blk + 4):
                    ps = auxp.tile([128, 512], f32, tag="aux", name="vps")
                    for k in range(KH):
                        nc.tensor.matmul(ps[:, 0:192], lhsT=xT[:, k, t * 128:(t + 1) * 128],
                                         rhs=wv[:, k, :], start=(k == 0), stop=False)
                    nc.tensor.matmul(ps[:, 0:192], lhsT=ones1[:], rhs=bv[:],
                                     start=False, stop=True)
                    nc.vector.tensor_scalar_mul(
                        vsb[:, t, :, 0:HD],
                        ps[:, 0:192].rearrange("p (h d) -> p h d", h=NH_CORE),
                        maskt[:, t:t + 1])
                    nc.vector.tensor_copy(vsb[:, t, :, HD:HD + 1],
                                          maskt[:, t:t + 1].to_broadcast((128, NH_CORE, 1)))

            def emit_attention(blk):
                base = blk * 512
                nkk = 4 * blk + 4
                oaccs = [accp.tile([HD + 1, 512], f32, tag="acc", name=f"oacc{_h}")
                         for _h in range(3)]

                def emit_av(kk, pt2, pt1, off):
                    for h, (pt, o2) in enumerate(((pt2, 0), (pt2, 512), (pt1, 0))):
                        nc.tensor.matmul(oaccs[h][:, off:512],
                                         lhsT=vsb[:, kk, h, :],
                                         rhs=pt[:, o2 + off:o2 + 512],
                                         start=(kk == 0), stop=(kk == nkk - 1),
                                         skip_group_check=True)

                pending = []   # AV emission lags 2 kk behind scores
                s1t = None
                pt1t = None
                for kk in range(nkk):
                    qlo = kk * 128
                    off = max(0, qlo - base)
                    # a kk-pair is "clean" when both halves are full-width:
                    # its two h2 exps fuse into one 1024-wide ACT op
                    pair_clean = (max(0, (kk | 1) * 128 - base) == 0)
                    s2 = mmp.tile([128, 1024], f32, tag="mm", name="s2")
                    nc.tensor.matmul(s2[:, off:512], lhsT=kTs[0][:, qlo:qlo + 128],
                                     rhs=qTs[0][:, base + off:base + 512],
                                     start=True, stop=True)
                    nc.tensor.matmul(s2[:, 512 + off:1024], lhsT=kTs[1][:, qlo:qlo + 128],
                                     rhs=qTs[1][:, base + off:base + 512],
                                     start=True, stop=True)
                    # h2 scores live in kk-pair halves of an mm-pool tile so
                    # the aux pool stays free of attention-phase rotation
                    if kk % 2 == 0:
                        s1t = mmp.tile([128, 1024], f32, tag="mm", name="s1t")
                        s1 = s1t[:, 0:512]
                        pt1t = ptp.tile([128, 1024], bf16, tag="pt1")
                        pt1 = pt1t[:, 0:512]
                    else:
                        s1 = s1t[:, 512:1024]
                        pt1 = pt1t[:, 512:1024]
                    nc.tensor.matmul(s1[:, off:512], lhsT=kTs[2][:, qlo:qlo + 128],
                                     rhs=qTs[2][:, base + off:base + 512],
                                     start=True, stop=True)
                    pt2 = ptp.tile([128, 1024], bf16, tag="pt2")
                    if off == 0:
                        nc.scalar.activation(out=pt2[:], in_=s2[:], func=Exp)
                    else:
                        nc.scalar.activation(out=pt2[:, off:512], in_=s2[:, off:512], func=Exp)
                        nc.scalar.activation(out=pt2[:, 512 + off:1024],
                                             in_=s2[:, 512 + off:1024], func=Exp)
                    if pair_clean:
                        if kk % 2 == 1:
                            nc.scalar.activation(out=pt1t[:], in_=s1t[:], func=Exp)
                    else:
                        nc.scalar.activation(out=pt1[:, off:512], in_=s1[:, off:512], func=Exp)
                    if qlo >= base:
                        d = off
                        for pt, o2 in ((pt2, 0), (pt2, 512), (pt1, 0)):
                            nc.vector.tensor_tensor(pt[:, o2 + d:o2 + d + 128],
                                                    pt[:, o2 + d:o2 + d + 128],
                                                    tri01[:], mybir.AluOpType.mult)
                    pending.append((kk, pt2, pt1, off))
                    if len(pending) > 2:
                        emit_av(*pending.pop(0))
                for p in pending:
                    emit_av(*p)
                return oaccs

            def emit_norm(blk, oaccs):
                base = blk * 512
                last = blk == NB - 1
                if last:
                    # nothing follows: read PSUM directly, shortest chain
                    srcs = oaccs
                else:
                    # stage oaccs to SBUF first: this releases the 3 oacc
                    # PSUM banks in ~2us so the next block's AV accumulation
                    # can start without waiting for the serial
                    # recip/broadcast/mult chain
                    srcs = []
                    for h in range(3):
                        oc = small.tile([HD + 1, 512], f32, tag=f"oc{h}", bufs=2)
                        nc.vector.tensor_copy(oc[:], oaccs[h][:])
                        srcs.append(oc)
                rrows = []
                for h in range(3):
                    # custom-DVE recip needs a partition-0-based SBUF input
                    srow = small.tile([1, 512], f32, tag="srow", bufs=3)
                    nc.vector.tensor_copy(srow[:], srcs[h][HD:HD + 1, :])
                    rrow = small.tile([1, 512], f32, tag="rrow", bufs=3)
                    nc.vector.reciprocal_approx_fast(rrow[:], srow[:])
                    rrows.append(rrow)
                for h in range(3):
                    rbs = small.tile([64, 512], f32, tag="rbs", bufs=3)
                    nc.gpsimd.partition_broadcast(rbs[:], rrows[h][:])
                    at_t, at_o = at_of[h]
                    nc.vector.tensor_tensor(at_t[at_o:at_o + HD, base:base + 512],
                                            srcs[h][0:HD, :], rbs[:],
                                            mybir.AluOpType.mult)

            def emit_proj(blk):
                # For blocks 0..NB-2 the proj overlaps the NEXT attention, so
                # it must NOT touch the mm pool (that would chain att(b+1)
                # behind norm(b)); it runs through aux-pool halves instead,
                # with yt copies on DVE (ACT is exp-bound during attention).
                # The last block runs after all attention: use the free mm
                # pool and the idle ACT engine.
                last = blk == NB - 1
                if last:
                    # head-pipelined: atA (h0/h1) matmuls issue as soon as
                    # their norm is done, atB (h2) follows — keeps the PE gap
                    # under the HAM re-throttle window at the tail
                    for tp in range(2):
                        tt = [4 * blk + 2 * tp, 4 * blk + 2 * tp + 1]
                        ypss = []
                        for t in tt:
                            ts = slice(t * 128, (t + 1) * 128)
                            yps = mmp.tile([128, 1024], f32, tag="mm", name="yps")
                            for ns in (slice(0, 512), slice(512, 768)):
                                nc.tensor.matmul(yps[:, ns], lhsT=atA[:, ts],
                                                 rhs=wp0[:, ns], start=True, stop=False)
                            ypss.append(yps)
                        for t, yps in zip(tt, ypss):
                            ts = slice(t * 128, (t + 1) * 128)
                            for ns in (slice(0, 512), slice(512, 768)):
                                nc.tensor.matmul(yps[:, ns], lhsT=atB[:, ts],
                                                 rhs=wp1[:, ns], start=False, stop=True)
                            yt = yout.tile([128, H], bf16)
                            nc.scalar.activation(out=yt[:], in_=yps[:, 0:768],
                                                 func=mybir.ActivationFunctionType.Copy)
                            nc.sync.dma_start(out=y_d.ap()[ts, :], in_=yt[:])
                    return
                for t in range(4 * blk, 4 * blk + 4):
                    ts = slice(t * 128, (t + 1) * 128)
                    yt = yout.tile([128, H], bf16)
                    for ns, w in ((slice(0, 512), 512), (slice(512, 768), 256)):
                        yps = auxp.tile([128, 512], f32, tag="aux", name="yps")
                        nc.tensor.matmul(yps[:, 0:w], lhsT=atA[:, ts], rhs=wp0[:, ns],
                                         start=True, stop=False)
                        nc.tensor.matmul(yps[:, 0:w], lhsT=atB[:, ts], rhs=wp1[:, ns],
                                         start=False, stop=True)
                        nc.vector.tensor_copy(yt[:, ns], yps[:, 0:w])
                    nc.sync.dma_start(out=y_d.ap()[ts, :], in_=yt[:])

            # qkT(b+1) is emitted between att(b) and norm(b), v(b+1) after
            # norm(b): the PE chews on next block's GEMMs while DVE/gpsimd
            # normalize block b, so the tensor engine never idles long enough
            # for HAM to re-throttle.
            emit_qkT(0)
            emit_v(0)
            for blk in range(NB):
                if blk + 1 < NB:
                    emit_xdma(blk + 1)
                oaccs = emit_attention(blk)
                if blk + 1 < NB:
                    emit_qkT(blk + 1)
                    emit_v(blk + 1)
                emit_norm(blk, oaccs)
                emit_proj(blk)

    nc.compile()
    return nc


def kernel(x, attn_mask, Wqkv, bqkv, Wproj, bproj):
    global last_results
    import ml_dtypes
    nbf16 = ml_dtypes.bfloat16
    x = np.asarray(x, dtype=np.float32)
    attn_mask = np.asarray(attn_mask)
    Wqkv = np.asarray(Wqkv, dtype=np.float32)
    bqkv = np.asarray(bqkv, dtype=np.float32)
    Wproj = np.asarray(Wproj, dtype=np.float32)
    bproj = np.asarray(bproj, dtype=np.float32)

    if "nc" not in _cache:
        _cache["nc"] = _build()
    nc = _cache["nc"]

    in_maps = []
    for c in range(8):
        b, g = c // 4, c % 4
        cs = slice(192 * g, 192 * g + 192)
        wq = Wqkv[:, 0:768][:, cs] * 0.125
        bq = bqkv[0:768][cs] * 0.125
        wk = Wqkv[:, 768:1536][:, cs]
        bk = bqkv[768:1536][cs]
        wvn = Wqkv[:, 1536:2304][:, cs]
        bvn = bqkv[1536:2304][cs]
        # chunks: c0=[q0|q1], c1=[k0|k1], c2=[q2|k2]
        wqk = np.concatenate([wq[:, 0:128], wk[:, 0:128],
                              wq[:, 128:192], wk[:, 128:192]], axis=1)  # [768, 384]
        bcat = np.concatenate([bq[0:128], bk[0:128],
                               bq[128:192], bk[128:192]]).astype(np.float32)
        bqk = np.ascontiguousarray(bcat.reshape(3, 128).T)  # [128, 3]
        maskt = np.ascontiguousarray(
            attn_mask[b].astype(np.float32).reshape(NT, 128).T)
        # device layouts: [128, KH, .] with row k*128+p at [p, k, :]
        wqk_p = np.ascontiguousarray(wqk.reshape(KH, 128, 384).transpose(1, 0, 2))
        wv_p = np.ascontiguousarray(wvn.reshape(KH, 128, 192).transpose(1, 0, 2))
        wp_p = np.zeros((128, 2, H), np.float32)
        wp_p[:, 0, :] = Wproj[cs, :][0:128]
        wp_p[0:64, 1, :] = Wproj[cs, :][128:192]
        in_maps.append({
            "xbT": np.ascontiguousarray(x[b].T).astype(nbf16),
            "wqk": wqk_p.astype(nbf16),
            "wv": wv_p.astype(nbf16),
            "wp": wp_p.astype(nbf16),
            "bqk": bqk,
            "bv": bvn.reshape(1, 192).astype(nbf16),
            "maskt": maskt,
        })

    trace = bool(int(os.environ.get("KERNEL_TRACE", "0")))
    res = run_bass_kernel_spmd(nc, in_maps, core_ids=list(range(8)), trace=trace)
    last_results = res

    parts = [res.results[c]["y"].astype(np.float32) for c in range(8)]
    out = np.stack([
        parts[0] + parts[1] + parts[2] + parts[3],
        parts[4] + parts[5] + parts[6] + parts[7],
    ]).astype(np.float32) + bproj.astype(np.float32)
    return out.astype(np.float32)


# revision 27
# speedup vs baseline: 1.0736x; 1.0736x over previous
"""Causal multi-head self-attention on 8 Trainium2 NeuronCores (Bass/Tile).

Problem (hardcoded shapes): x [2, 2048, 768] f32, 12 heads of dim 64.
    qkv = x @ Wqkv + bqkv ; per-head causal softmax(q k^T / 8) @ v ; out @ Wproj + bproj

Sharding: 8 cores = 2 batches x 4 head-groups (3 heads each). Each core computes
its heads' QKV, attention, and a partial output projection (its rows of Wproj).
Host sums the 4 partial projections per batch and adds bproj.

All matmul operands are bf16 (PSUM accumulation stays fp32); the 2e-2 rel-err
budget has ~20x headroom over the ~1e-3 this costs. bf16 enables FWL weight
loads and removes the fp32r staging/rounding passes entirely.

Single-pass structure over four 512-column blocks keeps the PE HAM-warm
(no >3.4us tensor-engine idle => stays at 2.4 GHz):
  per block b: DMA xT slice -> qkT chunks -> v tiles -> attention qc=b
               (scores S^T / exp / causal tri / AV, software-pipelined one kk
               deep against the ACT exp) -> normalize -> projection + DMA out.

Layouts per core (no on-device transposes needed anywhere):
  xT[128,6,2048]   x^T, host-pre-transposed, bf16
  qkc[128,3,2048]  [q0q1 | q2k0 | k1k2]^T, heads on 64-row slices (scale 1/8
                   folded into Wq on host)
  vsb[128,16,3,65] v natural + per-key mask column (ones here) for the
                   softmax denominator via the augmented-row trick
  S^T blocks [128 keys, <=512 queries] in PSUM; exp -> bf16 pt tiles; the
  key attn_mask is folded into vsb (mask*v rows and mask sum-column), which
  is exactly equivalent to -inf score masking, so exp needs no bias at all.
  oacc[65,512] fp32 accumulates v_aug^T P^T over key tiles; row 64 = softmax
  denominator; normalize via fast reciprocal + gpsimd partition broadcast.
  y = A @ Wproj emitted bf16; host sums the 4 partials per batch in fp32.

PSUM budget (8 banks): s2/proj pool [128,1024]x2 = 4, aux [128,512]x1 = 1,
oacc [65,512]x3 = 3.
"""
import os
import numpy as np

import concourse.bass as bass
import concourse.mybir as mybir
import concourse.tile as tile
from concourse import bacc
from concourse.bass_utils import run_bass_kernel_spmd
from concourse.masks import make_upper_triangular

f32 = mybir.dt.float32
bf16 = mybir.dt.bfloat16

T = 2048          # sequence length
H = 768           # model dim
NH_CORE = 3       # heads per core
HD = 64           # head dim
NT = T // 128     # 16 seq tiles
KH = H // 128     # 6 contraction chunks for H
NB = 4            # 512-wide column blocks

_cache = {}
last_results = None


def _build():
    """Build the single-core Tile program (same program on all 8 cores)."""
    nc = bacc.Bacc("TRN2", target_bir_lowering=False, debug=False)

    x_d = nc.dram_tensor("xbT", [H, T], bf16, kind="ExternalInput")
    # weights host-prepacked to [128, ...] so each DMA line is one contiguous
    # multi-KB run per partition (row-major [H, N] layouts generate 768-byte
    # descriptors that clog the queues for ~25us at startup)
    wqk_d = nc.dram_tensor("wqk", [128, KH, 384], bf16, kind="ExternalInput")
    wv_d = nc.dram_tensor("wv", [128, KH, 192], bf16, kind="ExternalInput")
    wp_d = nc.dram_tensor("wp", [128, 2, H], bf16, kind="ExternalInput")
    bqk_d = nc.dram_tensor("bqk", [128, 3], f32, kind="ExternalInput")
    bv_d = nc.dram_tensor("bv", [1, 192], bf16, kind="ExternalInput")
    mask_d = nc.dram_tensor("maskt", [128, NT], f32, kind="ExternalInput")
    y_d = nc.dram_tensor("y", [T, H], bf16, kind="ExternalOutput")

    Exp = mybir.ActivationFunctionType.Exp

    with tile.TileContext(nc) as tc:
        with (
            tc.tile_pool(name="singles", bufs=1) as singles,
            tc.tile_pool(name="big", bufs=1) as big,
            tc.tile_pool(name="pt", bufs=4) as ptp,
            tc.tile_pool(name="yout", bufs=2) as yout,
            tc.tile_pool(name="small", bufs=2) as small,
            tc.tile_pool(name="mm", bufs=2, space="PSUM") as mmp,
            tc.tile_pool(name="aux", bufs=1, space="PSUM") as auxp,
            tc.tile_pool(name="acc", bufs=3, space="PSUM") as accp,
        ):
            # ---- constants / weights ----
            tri_f = singles.tile([128, 128], f32)
            make_upper_triangular(nc, tri_f[:], val=1.0, diag=True)
            tri01 = singles.tile([128, 128], bf16)
            nc.vector.tensor_copy(tri01[:], tri_f[:])
            ones1 = singles.tile([1, 128], bf16)
            nc.vector.memset(ones1[:], 1.0)

            xT = big.tile([128, KH, T], bf16)
            # interleave wqk / xT-block0 loads per k-chunk so the first qkT
            # matmul only waits for ~0.2MB of DMA, not the full weight set
            wqk = singles.tile([128, KH, 384], bf16)
            nc.sync.dma_start(out=wqk[:], in_=wqk_d.ap())
            for k in range(KH):
                nc.sync.dma_start(out=xT[:, k, 0:512], in_=x_d.ap()[k * 128:(k + 1) * 128, 0:512])
            bqk = singles.tile([128, 3], f32)
            nc.sync.dma_start(out=bqk[:], in_=bqk_d.ap())
            wv = singles.tile([128, KH, 192], bf16)
            nc.sync.dma_start(out=wv[:], in_=wv_d.ap())
            bv = singles.tile([1, 192], bf16)
            nc.sync.dma_start(out=bv[:], in_=bv_d.ap())
            maskt = singles.tile([128, NT], f32)
            nc.sync.dma_start(out=maskt[:], in_=mask_d.ap())
            wpb = singles.tile([128, 2, H], bf16)
            nc.sync.dma_start(out=wpb[:], in_=wp_d.ap())
            wp0 = wpb[:, 0, :]
            wp1 = wpb[0:64, 1, :]
            qkc = big.tile([128, 3, T], bf16)
            vsb = big.tile([128, NT, NH_CORE, HD + 1], bf16)
            atA = big.tile([128, T], bf16)   # A^T for h0 (0:64), h1 (64:128)
            atB = big.tile([64, T], bf16)    # h2
            at_of = [(atA, 0), (atA, 64), (atB, 0)]
            # chunks: c0=[q0|q1], c1=[k0|k1], c2=[q2|k2]; matmul requires
            # lhsT/rhs base partitions to match, so k2 gets realigned to
            # base 0 via a small per-block copy into qkE.
            qkE = big.tile([64, T], bf16)
            qTs = [qkc[0:64, 0, :], qkc[64:128, 0, :], qkc[0:64, 2, :]]
            kTs = [qkc[0:64, 1, :], qkc[64:128, 1, :], qkE[:, :]]

            def emit_xdma(blk):
                bs = slice(blk * 512, (blk + 1) * 512)
                for k in range(KH):
                    nc.sync.dma_start(out=xT[:, k, bs],
                                      in_=x_d.ap()[k * 128:(k + 1) * 128, bs])

            def emit_qkT(blk):
                bs = slice(blk * 512, (blk + 1) * 512)
                # qkT chunks [384, 512]
                for c in range(3):
                    ps = auxp.tile([128, 512], f32, tag="aux", name="qkps")
                    for k in range(KH):
                        nc.tensor.matmul(ps[:], lhsT=wqk[:, k, c * 128:(c + 1) * 128],
                                         rhs=xT[:, k, bs], start=(k == 0), stop=(k == KH - 1))
                    nc.vector.tensor_scalar_add(qkc[:, c, bs], ps[:], bqk[:, c:c + 1])
                nc.vector.tensor_copy(qkE[:, bs], qkc[64:128, 2, bs])

            def emit_v(blk):
                # v natural for 4 t-tiles, mask folded in
                for t in range(4 * blk, 4 * blk + 4):
                    ps = auxp.tile([128, 512], f32, tag="aux", name="vps")
                    for k in range(KH):
                        nc.tensor.matmul(ps[:, 0:192], lhsT=xT[:, k, t * 128:(t + 1) * 128],
                                         rhs=wv[:, k, :], start=(k == 0), stop=False)
                    nc.tensor.matmul(ps[:, 0:192], lhsT=ones1[:], rhs=bv[:],
                                     start=False, stop=True)
                    nc.vector.tensor_scalar_mul(
                        vsb[:, t, :, 0:HD],
                        ps[:, 0:192].rearrange("p (h d) -> p h d", h=NH_CORE),
                        maskt[:, t:t + 1])
                    nc.vector.tensor_copy(vsb[:, t, :, HD:HD + 1],
                                          maskt[:, t:t + 1].to_broadcast((128, NH_CORE, 1)))

            def emit_attention(blk):
                base = blk * 512
                nkk = 4 * blk + 4
                oaccs = [accp.tile([HD + 1, 512], f32, tag="acc", name=f"oacc{_h}")
                         for _h in range(3)]

                def emit_av(kk, pt2, pt1, off):
                    for h, (pt, o2) in enumerate(((pt2, 0), (pt2, 512), (pt1, 0))):
                        nc.tensor.matmul(oaccs[h][:, off:512],
                                         lhsT=vsb[:, kk, h, :],
                                         rhs=pt[:, o2 + off:o2 + 512],
                                         start=(kk == 0), stop=(kk == nkk - 1),
                                         skip_group_check=True)

                pending = []   # AV emission lags 2 kk behind scores
                s1t = None
                pt1t = None
                for kk in range(nkk):
                    qlo = kk * 128
                    off = max(0, qlo - base)
                    # a kk-pair is "clean" when both halves are full-width:
                    # its two h2 exps fuse into one 1024-wide ACT op
                    pair_clean = (max(0, (kk | 1) * 128 - base) == 0)
                    s2 = mmp.tile([128, 1024], f32, tag="mm", name="s2")
                    nc.tensor.matmul(s2[:, off:512], lhsT=kTs[0][:, qlo:qlo + 128],
                                     rhs=qTs[0][:, base + off:base + 512],
                                     start=True, stop=True)
                    nc.tensor.matmul(s2[:, 512 + off:1024], lhsT=kTs[1][:, qlo:qlo + 128],
                                     rhs=qTs[1][:, base + off:base + 512],
                                     start=True, stop=True)
                    # h2 scores live in kk-pair halves of an mm-pool tile so
                    # the aux pool stays free of attention-phase rotation
                    if kk % 2 == 0:
                        s1t = mmp.tile([128, 1024], f32, tag="mm", name="s1t")
                        s1 = s1t[:, 0:512]
                        pt1t = ptp.tile([128, 1024], bf16, tag="pt1")
                        pt1 = pt1t[:, 0:512]
                    else:
                        s1 = s1t[:, 512:1024]
                        pt1 = pt1t[:, 512:1024]
                    nc.tensor.matmul(s1[:, off:512], lhsT=kTs[2][:, qlo:qlo + 128],
                                     rhs=qTs[2][:, base + off:base + 512],
                                     start=True, stop=True)
                    pt2 = ptp.tile([128, 1024], bf16, tag="pt2")
                    if off == 0:
                        nc.scalar.activation(out=pt2[:], in_=s2[:], func=Exp)
                    else:
                        nc.scalar.activation(out=pt2[:, off:512], in_=s2[:, off:512], func=Exp)
                        nc.scalar.activation(out=pt2[:, 512 + off:1024],
                                             in_=s2[:, 512 + off:1024], func=Exp)
                    if pair_clean:
                        if kk % 2 == 1:
                            nc.scalar.activation(out=pt1t[:], in_=s1t[:], func=Exp)
                    else:
                        nc.scalar.activation(out=pt1[:, off:512], in_=s1[:, off:512], func=Exp)
                    if qlo >= base:
                        d = off
                        for pt, o2 in ((pt2, 0), (pt2, 512), (pt1, 0)):
                            nc.vector.tensor_tensor(pt[:, o2 + d:o2 + d + 128],
                                                    pt[:, o2 + d:o2 + d + 128],
                                                    tri01[:], mybir.AluOpType.mult)
                    pending.append((kk, pt2, pt1, off))
                    if len(pending) > 1:
                        emit_av(*pending.pop(0))
                for p in pending:
                    emit_av(*p)
                return oaccs

            def emit_norm(blk, oaccs):
                base = blk * 512
                last = blk == NB - 1
                if last:
                    # nothing follows: read PSUM directly, shortest chain
                    srcs = oaccs
                else:
                    # stage oaccs to SBUF first: this releases the 3 oacc
                    # PSUM banks in ~2us so the next block's AV accumulation
                    # can start without waiting for the serial
                    # recip/broadcast/mult chain
                    srcs = []
                    for h in range(3):
                        oc = small.tile([HD + 1, 512], f32, tag=f"oc{h}", bufs=2)
                        nc.vector.tensor_copy(oc[:], oaccs[h][:])
                        srcs.append(oc)
                rrows = []
                for h in range(3):
                    # custom-DVE recip needs a partition-0-based SBUF input
                    srow = small.tile([1, 512], f32, tag="srow", bufs=3)
                    nc.vector.tensor_copy(srow[:], srcs[h][HD:HD + 1, :])
                    rrow = small.tile([1, 512], f32, tag="rrow", bufs=3)
                    nc.vector.reciprocal_approx_fast(rrow[:], srow[:])
                    rrows.append(rrow)
                for h in range(3):
                    rbs = small.tile([64, 512], f32, tag="rbs", bufs=3)
                    nc.gpsimd.partition_broadcast(rbs[:], rrows[h][:])
                    at_t, at_o = at_of[h]
                    nc.vector.tensor_tensor(at_t[at_o:at_o + HD, base:base + 512],
                                            srcs[h][0:HD, :], rbs[:],
                                            mybir.AluOpType.mult)

            def emit_proj(blk):
                # For blocks 0..NB-2 the proj overlaps the NEXT attention, so
                # it must NOT touch the mm pool (that would chain att(b+1)
                # behind norm(b)); it runs through aux-pool halves instead,
                # with yt copies on DVE (ACT is exp-bound during attention).
                # The last block runs after all attention: use the free mm
                # pool and the idle ACT engine.
                last = blk == NB - 1
                if last:
                    # head-pipelined: atA (h0/h1) matmuls issue as soon as
                    # their norm is done, atB (h2) follows — keeps the PE gap
                    # under the HAM re-throttle window at the tail
                    for tp in range(2):
                        tt = [4 * blk + 2 * tp, 4 * blk + 2 * tp + 1]
                        ypss = []
                        for t in tt:
                            ts = slice(t * 128, (t + 1) * 128)
                            yps = mmp.tile([128, 1024], f32, tag="mm", name="yps")
                            for ns in (slice(0, 512), slice(512, 768)):
                                nc.tensor.matmul(yps[:, ns], lhsT=atA[:, ts],
                                                 rhs=wp0[:, ns], start=True, stop=False)
                            ypss.append(yps)
                        for t, yps in zip(tt, ypss):
                            ts = slice(t * 128, (t + 1) * 128)
                            for ns in (slice(0, 512), slice(512, 768)):
                                nc.tensor.matmul(yps[:, ns], lhsT=atB[:, ts],
                                                 rhs=wp1[:, ns], start=False, stop=True)
                            yt = yout.tile([128, H], bf16)
                            nc.scalar.activation(out=yt[:], in_=yps[:, 0:768],
                                                 func=mybir.ActivationFunctionType.Copy)
                            nc.sync.dma_start(out=y_d.ap()[ts, :], in_=yt[:])
                    return
                for t in range(4 * blk, 4 * blk + 4):
                    ts = slice(t * 128, (t + 1) * 128)
                    yt = yout.tile([128, H], bf16)
                    for ns, w in ((slice(0, 512), 512), (slice(512, 768), 256)):
                        yps = auxp.tile([128, 512], f32, tag="aux", name="yps")
                        nc.tensor.matmul(yps[:, 0:w], lhsT=atA[:, ts], rhs=wp0[:, ns],
                                         start=True, stop=False)
                        nc.tensor.matmul(yps[:, 0:w], lhsT=atB[:, ts], rhs=wp1[:, ns],
                                         start=False, stop=True)
                        nc.vector.tensor_copy(yt[:, ns], yps[:, 0:w])
                    nc.sync.dma_start(out=y_d.ap()[ts, :], in_=yt[:])

            # qkT(b+1) is emitted between att(b) and norm(b), v(b+1) after
            # norm(b): the PE chews on next block's GEMMs while DVE/gpsimd
            # normalize block b, so the tensor engine never idles long enough
            # for HAM to re-throttle.
            emit_qkT(0)
            emit_v(0)
            for blk in range(NB):
                if blk + 1 < NB:
                    emit_xdma(blk + 1)
                oaccs = emit_attention(blk)
                if blk + 1 < NB:
                    emit_qkT(blk + 1)
                    emit_v(blk + 1)
                emit_norm(blk, oaccs)
                emit_proj(blk)

    nc.compile()
    return nc


def kernel(x, attn_mask, Wqkv, bqkv, Wproj, bproj):
    global last_results
    import ml_dtypes
    nbf16 = ml_dtypes.bfloat16
    x = np.asarray(x, dtype=np.float32)
    attn_mask = np.asarray(attn_mask)
    Wqkv = np.asarray(Wqkv, dtype=np.float32)
    bqkv = np.asarray(bqkv, dtype=np.float32)
    Wproj = np.asarray(Wproj, dtype=np.float32)
    bproj = np.asarray(bproj, dtype=np.float32)

    if "nc" not in _cache:
        _cache["nc"] = _build()
    nc = _cache["nc"]

    in_maps = []
    for c in range(8):
        b, g = c // 4, c % 4
        cs = slice(192 * g, 192 * g + 192)
        wq = Wqkv[:, 0:768][:, cs] * 0.125
        bq = bqkv[0:768][cs] * 0.125
        wk = Wqkv[:, 768:1536][:, cs]
        bk = bqkv[768:1536][cs]
        wvn = Wqkv[:, 1536:2304][:, cs]
        bvn = bqkv[1536:2304][cs]
        # chunks: c0=[q0|q1], c1=[k0|k1], c2=[q2|k2]
        wqk = np.concatenate([wq[:, 0:128], wk[:, 0:128],
                              wq[:, 128:192], wk[:, 128:192]], axis=1)  # [768, 384]
        bcat = np.concatenate([bq[0:128], bk[0:128],
                               bq[128:192], bk[128:192]]).astype(np.float32)
        bqk = np.ascontiguousarray(bcat.reshape(3, 128).T)  # [128, 3]
        maskt = np.ascontiguousarray(
            attn_mask[b].astype(np.float32).reshape(NT, 128).T)
        # device layouts: [128, KH, .] with row k*128+p at [p, k, :]
        wqk_p = np.ascontiguousarray(wqk.reshape(KH, 128, 384).transpose(1, 0, 2))
        wv_p = np.ascontiguousarray(wvn.reshape(KH, 128, 192).transpose(1, 0, 2))
        wp_p = np.zeros((128, 2, H), np.float32)
        wp_p[:, 0, :] = Wproj[cs, :][0:128]
        wp_p[0:64, 1, :] = Wproj[cs, :][128:192]
        in_maps.append({
            "xbT": np.ascontiguousarray(x[b].T).astype(nbf16),
            "wqk": wqk_p.astype(nbf16),
            "wv": wv_p.astype(nbf16),
            "wp": wp_p.astype(nbf16),
            "bqk": bqk,
            "bv": bvn.reshape(1, 192).astype(nbf16),
            "maskt": maskt,
        })

    trace = bool(int(os.environ.get("KERNEL_TRACE", "0")))
    res = run_bass_kernel_spmd(nc, in_maps, core_ids=list(range(8)), trace=trace)
    last_results = res

    parts = [res.results[c]["y"].astype(np.float32) for c in range(8)]
    out = np.stack([
        parts[0] + parts[1] + parts[2] + parts[3],
        parts[4] + parts[5] + parts[6] + parts[7],
    ]).astype(np.float32) + bproj.astype(np.float32)
    return out.astype(np.float32)
